# revision 29
# baseline (speedup 1.0000x reference)
"""Trainium2 Bass kernel for nn_Attention_62861141344964.

Full-input contract: kernel(**inputs) takes the unsharded inputs and returns
the full-shape output. Internally shards across 8 NeuronCores as
(batch, head-pair): core c handles batch c//4 and heads {2*(c%4), 2*(c%4)+1}.

Per-core pipeline (ACT-exp is the bottleneck engine; everything else is
arranged around keeping its exp stream dense):
  - prologue per n-block: x DMA -> xsq(fp8, DVE) -> sumsq (fp8 DoubleRow
    matmul) -> DMA round-trip to transposed layout -> rsqrt via DVE bit-trick
    + 2 Newton steps (no ACT sqrt, so ACT runs exp only, one table load) ->
    broadcast (Pool) -> q/k projections (f32r) -> q2/k2 scaled (DVE).
  - attention: sim matmuls f32r -> exp on ACT with bias -2 emitting fp8e4
    directly -> AV as fp8 DoubleRow over j-tile pairs (0.5 cyc/row, two
    j-tiles per instruction) with plain-fp8 orphans; denominator rides row 0
    of vT/av (ones trick).
  - tail per i-block: reciprocal(den) -> Pool partition_broadcast -> DVE
    normalize into `on` (fp8) -> output projection as one fp8-DoubleRow
    matmul per 128-chunk (both heads contracted together) -> residual add on
    Pool -> DMA out.
The host folds g*sqrt(c) (and q's 1/8) into the weights and sums the 4
partial outputs per batch.
"""

import sys

sys.path.insert(0, "/opt/trn_rl_repo")

import numpy as np

HEADS = 8
DH = 64
DIM = 512
B = 2
HWS = 48
N = HWS * HWS  # 2304
KT = 4  # k-tiles of 128 over DIM
JT = 18  # j-tiles of 128 over N
NBLKS = [(0, 512), (512, 512), (1024, 512), (1536, 512), (2048, 256)]
MAGIC_SQRT = 0x1FBD1DF5
SQRT32 = 5.656854249492381

_CACHE = {}


def _build_program(debug=False):
    import concourse.bass as bass  # noqa: F401
    import concourse.mybir as mybir
    import concourse.tile as tile
    from concourse import bacc

    f32 = mybir.dt.float32
    f32r = mybir.dt.float32r
    bf16 = mybir.dt.bfloat16
    f8 = mybir.dt.float8e4
    i32 = mybir.dt.int32
    AF = mybir.ActivationFunctionType
    OP = mybir.AluOpType
    DRM = mybir.MatmulPerfMode.DoubleRow

    nc = bacc.Bacc("TRN2", target_bir_lowering=False, debug=False, num_devices=8)

    x4_d = nc.dram_tensor("x4", [DIM, N], f32r, kind="ExternalInput").ap()
    xb_d = nc.dram_tensor("xbin", [DIM, N], bf16, kind="ExternalInput").ap()
    wqk_d = nc.dram_tensor("wqk", [DIM, 256], f32r, kind="ExternalInput").ap()
    wv_d = nc.dram_tensor("wv", [DIM, 128], bf16, kind="ExternalInput").ap()
    wp_d = nc.dram_tensor("wp", [65, 2, DIM], f8, kind="ExternalInput").ap()
    y_d = nc.dram_tensor("y", [DIM, N], f32, kind="ExternalOutput").ap()
    dbg = {}
    if debug:
        for nm, shp, dt in [("q2", [128, N], f32), ("k2", [128, N], f32),
                            ("vT", [128, JT, 144], f32), ("on", [65, 2, N], f32),
                            ("s_bc", [128, N], f32), ("s_colT", [128, JT], f32)]:
            dbg[nm] = nc.dram_tensor("dbg_" + nm, shp, dt, kind="ExternalOutput").ap()

    with tile.TileContext(nc) as tc:
        big = tc.alloc_tile_pool(name="big", bufs=1)
        work = tc.alloc_tile_pool(name="work", bufs=2)
        rsq = tc.alloc_tile_pool(name="rsq", bufs=6)
        pg = tc.alloc_tile_pool(name="pg", bufs=1, space="PSUM")
        pav = tc.alloc_tile_pool(name="pav", bufs=3, space="PSUM")

        # ---------- persistent tiles ----------
        x4s = big.tile([128, KT, N], f32r)
        xb = big.tile([128, KT, N], bf16)
        q2 = big.tile([128, N], f32r)
        k2 = big.tile([128, N], f32r)
        s_bc = big.tile([128, N], f32)
        s_row = big.tile([1, N], f32)
        t_colT = big.tile([128, JT], f32)
        s_colT = big.tile([128, JT], f32)
        vT = big.tile([128, JT, 144], f8)
        on = big.tile([65, 2, N], f8)
        wqk_s = big.tile([128, KT, 256], f32r)
        wv_s = big.tile([128, KT, 128], bf16)
        wp_s = big.tile([65, 2, DIM], f8)
        ones8 = big.tile([128, 2, 16], f8)
        bias_m2 = big.tile([128, 1], f32)
        e64 = big.tile([128, 128], f32r)  # row 64 = ones: PE partition-bcast of row 64
        den_pad = [big.tile([128, 512], f32r, name="den_pad0"),
                   big.tile([128, 512], f32r, name="den_pad1")]


        nc.gpsimd.memset(ones8[:], 1.0)
        nc.vector.memset(bias_m2[:], -2.0)
        nc.gpsimd.memset(vT[:, :, 64:65], 1.0)
        nc.gpsimd.memset(vT[:, :, 136:137], 1.0)
        nc.vector.memset(e64[:].bitcast(f32), 0.0)
        nc.vector.memset(e64[64:65, :].bitcast(f32), 1.0)

        # x loads split across DMA rings so no single queue serializes them;
        # x block 1 leads the ACT ring (ahead of weights + the bf16 x copy),
        # x block 3 is issued on the Pool ring after the eager block-0
        # prologue so its trigger doesn't block the s broadcast
        x4_r = x4_d.rearrange("(a p) n -> p a n", p=128)
        xb_r = xb_d.rearrange("(a p) n -> p a n", p=128)
        def x_load(bi, eng):
            o, w = NBLKS[bi]
            eng.dma_start(x4s[:, :, o : o + w], x4_r[:, :, o : o + w])
        x_load(0, nc.sync)
        x_load(1, nc.scalar)
        x_load(3, nc.gpsimd)
        x_load(2, nc.sync)
        x_load(4, nc.sync)
        nc.scalar.dma_start(wqk_s[:], wqk_d.rearrange("(a p) m -> p a m", p=128))
        nc.scalar.dma_start(wv_s[:], wv_d.rearrange("(a p) m -> p a m", p=128))
        nc.scalar.dma_start(wp_s[:], wp_d)
        for (o, w) in NBLKS:
            nc.scalar.dma_start(xb[:, :, o : o + w], xb_r[:, :, o : o + w])

        # ---------- prologue jobs (per n-block) ----------
        # The per-token norm scale s = sqrt(32/sumsq) is needed in TWO
        # layouts: as a row (-> partition_broadcast -> s_bc, scaling q2/k2
        # along the free axis) and transposed (s_colT, per-partition scalar
        # for the vT scaling). Computing sumsq in both orientations on the
        # PE (ones-vector matmuls) and running the rsqrt bit-trick + Newton
        # on each avoids any DMA transpose round-trip.
        def pro_sumsq(bi):
            o, w = NBLKS[bi]
            c0, cw = o // 128, w // 128

            def job():
                xsq = work.tile([128, KT, 512], f8, tag="xsq", name=f"xsq_{bi}")
                ps = pav.tile([1, 512], f32, tag="avy", name=f"ps_{bi}")
                for k in range(2):
                    nc.vector.tensor_tensor(
                        xsq[:, 2 * k : 2 * k + 2, :w],
                        x4s[:, 2 * k : 2 * k + 2, o : o + w],
                        x4s[:, 2 * k : 2 * k + 2, o : o + w], OP.mult,
                    )
                    nc.tensor.matmul(
                        ps[:, :w], ones8[:, :, 0:1], xsq[:, 2 * k : 2 * k + 2, :w],
                        start=(k == 0), stop=(k == 1), perf_mode=DRM,
                    )
                pt = pav.tile([128, 512], f32, tag="avy", name=f"pt_{bi}")
                for tt in range(cw):
                    for k in range(2):
                        nc.tensor.matmul(
                            pt[:, tt : tt + 1],
                            xsq[:, 2 * k : 2 * k + 2, tt * 128 : (tt + 1) * 128],
                            ones8[:, :, 0:1],
                            start=(tt == 0 and k == 0),
                            stop=(tt == cw - 1 and k == 1),
                            perf_mode=DRM,
                            skip_group_check=True,
                        )
                # row rsqrt: s_row = sqrt(32/t), seed from bits of 1/t,
                # one Newton step (0.2% worst case). t is copied to SBUF
                # first so the ps PSUM bank frees immediately.
                r0 = work.tile([1, 512], f32, tag="r0", name=f"r0_{bi}")
                r1 = work.tile([1, 512], f32, tag="r1", name=f"r1_{bi}")
                r2 = work.tile([1, 512], f32, tag="r2", name=f"r2_{bi}")
                t_sb = work.tile([1, 512], f32, tag="tsb", name=f"tsb_{bi}")
                nc.vector.tensor_copy(t_sb[:, :w], ps[:, :w])
                nc.vector.reciprocal(r0[:, :w], t_sb[:, :w])
                nc.vector.tensor_scalar(
                    r0[:, :w].bitcast(i32), r0[:, :w].bitcast(i32), 1, None,
                    OP.logical_shift_right,
                )
                nc.vector.tensor_scalar(
                    r0[:, :w].bitcast(i32), r0[:, :w].bitcast(i32), MAGIC_SQRT,
                    None, OP.add,
                )
                nc.vector.tensor_tensor(r1[:, :w], r0[:, :w], r0[:, :w], OP.mult)
                nc.vector.tensor_tensor(r2[:, :w], r1[:, :w], t_sb[:, :w], OP.mult)
                nc.vector.tensor_scalar(
                    r2[:, :w], r2[:, :w], -0.5 * SQRT32, 1.5 * SQRT32, OP.mult, OP.add
                )
                nc.vector.tensor_tensor(s_row[:, o : o + w], r2[:, :w], r0[:, :w], OP.mult)
                nc.gpsimd.partition_broadcast(s_bc[:, o : o + w], s_row[:, o : o + w])
                # transposed rsqrt for s_colT (tiny frees; two Newton steps)
                tc_ = rsq.tile([128, 4], f32, tag="rsq", name=f"tc_{bi}")
                nc.vector.tensor_copy(tc_[:, :cw], pt[:, :cw])
                y0 = rsq.tile([128, 4], f32, tag="rsq", name=f"y0_{bi}")
                aa = rsq.tile([128, 4], f32, tag="rsq", name=f"aa_{bi}")
                bb = rsq.tile([128, 4], f32, tag="rsq", name=f"bb_{bi}")
                nc.vector.reciprocal(y0[:, :cw], tc_[:, :cw])
                nc.vector.tensor_scalar(
                    y0[:, :cw].bitcast(i32), y0[:, :cw].bitcast(i32), 1, None,
                    OP.logical_shift_right,
                )
                nc.vector.tensor_scalar(
                    y0[:, :cw].bitcast(i32), y0[:, :cw].bitcast(i32), MAGIC_SQRT,
                    None, OP.add,
                )
                nc.vector.tensor_tensor(aa[:, :cw], y0[:, :cw], y0[:, :cw], OP.mult)
                nc.vector.tensor_tensor(bb[:, :cw], aa[:, :cw], tc_[:, :cw], OP.mult)
                nc.vector.tensor_scalar(
                    aa[:, :cw], bb[:, :cw], -0.5 * SQRT32, 1.5 * SQRT32, OP.mult, OP.add
                )
                nc.vector.tensor_tensor(
                    s_colT[:, c0 : c0 + cw], aa[:, :cw], y0[:, :cw], OP.mult
                )
            return job

        def pro_k(bi):
            o, w = NBLKS[bi]

            def job():
                pk = pav.tile([128, 512], f32, tag="avy", name=f"pk_{bi}")
                for kt in range(KT):
                    nc.tensor.matmul(
                        pk[:, :w], wqk_s[:, kt, 128:256], x4s[:, kt, o : o + w],
                        start=(kt == 0), stop=(kt == KT - 1),
                    )
                nc.vector.tensor_tensor(
                    k2[:, o : o + w], pk[:, :w], s_bc[:, o : o + w], OP.mult
                )
            return job

        def pro_q(bi):
            o, w = NBLKS[bi]

            def job():
                pq = pav.tile([128, 512], f32, tag="avy", name=f"pq_{bi}")
                for kt in range(KT):
                    nc.tensor.matmul(
                        pq[:, :w], wqk_s[:, kt, 0:128], x4s[:, kt, o : o + w],
                        start=(kt == 0), stop=(kt == KT - 1),
                    )
                nc.vector.tensor_tensor(
                    q2[:, o : o + w], pq[:, :w], s_bc[:, o : o + w], OP.mult
                )
            return job

        def vt_job(jt):
            def job():
                pv = pav.tile([128, 512], f32, tag="avy", name=f"pv_{jt}")
                for kt in range(KT):
                    nc.tensor.matmul(
                        pv[:, :128], xb[:, kt, jt * 128 : (jt + 1) * 128],
                        wv_s[:, kt, :], start=(kt == 0), stop=(kt == KT - 1),
                    )
                nc.vector.tensor_scalar_mul(
                    vT[:, jt, 0:64], pv[:, 0:64], s_colT[:, jt : jt + 1]
                )
                nc.vector.tensor_scalar_mul(
                    vT[:, jt, 72:136], pv[:, 64:128], s_colT[:, jt : jt + 1]
                )
            return job

        # job queue: prologue for blocks 1.. interleaved with vT jobs, popped
        # during the attention waves (block 0's prologue is emitted eagerly)
        jobs = []
        pro_done = [False] * len(NBLKS)
        vt_done = [False] * JT

        def mark_pro(bi):
            def f():
                pro_done[bi] = True
            return f

        def mark_vt(jt):
            def f():
                vt_done[jt] = True
            return f

        for fn in (pro_sumsq(0), pro_k(0), pro_q(0)):
            fn()
        pro_done[0] = True

        def late_memsets():
            nc.gpsimd.memset(on[64:65, :, :], 1.0)
            nc.gpsimd.memset(den_pad[0][:].bitcast(f32), 0.0)
            nc.gpsimd.memset(den_pad[1][:].bitcast(f32), 0.0)

        for bi in range(1, len(NBLKS)):
            jobs += [pro_sumsq(bi), pro_k(bi),
                     (pro_q(bi), mark_pro(bi))]
            lo = 4 * (bi - 1)
            jobs += [(vt_job(jt), mark_vt(jt)) for jt in range(lo, min(lo + 4, JT))]
        jobs.append(late_memsets)
        jobs += [(vt_job(jt), mark_vt(jt)) for jt in range(16, JT)]

        def pop_job():
            j = jobs.pop(0)
            if isinstance(j, tuple):
                j[0]()
                j[1]()
            else:
                j()

        def ensure_vt(jt):
            while not vt_done[jt]:
                pop_job()

        def ensure_block(bi):
            while not pro_done[bi]:
                pop_job()

        pwav = tc.alloc_tile_pool(name="pwav", bufs=10)
        ywork = tc.alloc_tile_pool(name="ywork", bufs=3)

        # ---------- wave plans ----------
        # full blocks (w=512): alternating G3/G2 tiles of (head, [jts]);
        # 3-waves carry a DR pair + an orphan, 2-waves a DR pair.
        # one head at a time per i-block: only one av accumulator is live,
        # so two of the three avy PSUM slots stay free for prologue/vT/tail
        # transients (two n-block prologue chains can be in flight)
        def wave_plan_512():
            plan = []  # (gtag, size, head, jts)
            for head in (0, 1):
                jts = list(range(JT))
                for size in (3, 2, 3, 2, 3, 2, 3):
                    plan.append(("G3" if size == 3 else "G2", size, head,
                                 [jts.pop(0) for _ in range(size)]))
                assert not jts
            return plan

        def wave_plan_256():
            plan = []
            for head in (0, 1):
                jts = list(range(JT))
                for size in (6, 4, 6, 2):
                    plan.append(("G3" if size == 6 else "G2", size, head,
                                 [jts.pop(0) for _ in range(size)]))
                assert not jts
            return plan

        def emit_sims(g, o, w, head, jts):
            for slot, jt in enumerate(jts):
                nc.tensor.matmul(
                    g[:, slot, :],
                    k2[64 * head : 64 * (head + 1), jt * 128 : (jt + 1) * 128],
                    q2[64 * head : 64 * (head + 1), o : o + w],
                    start=True, stop=True,
                )

        def emit_avs(psb, w, head, jts, av, flags):
            # DR pairs over consecutive slots, plain fp8 for the odd orphan
            vbase = 72 * head
            i = 0
            while i < len(jts):
                first = flags["first"]
                if i + 1 < len(jts) and jts[i + 1] == jts[i] + 1:
                    last = flags["remaining"] == 2
                    nc.tensor.matmul(
                        av[:, :w],
                        vT[:, jts[i] : jts[i] + 2, vbase : vbase + 65],
                        psb[:, i : i + 2, :],
                        start=first, stop=last, perf_mode=DRM,
                        skip_group_check=True,
                    )
                    i += 2
                    flags["remaining"] -= 2
                else:
                    last = flags["remaining"] == 1
                    nc.tensor.matmul(
                        av[:, :w],
                        vT[:, jts[i], vbase : vbase + 65],
                        psb[:, i, :],
                        start=first, stop=last, skip_group_check=True,
                    )
                    i += 1
                    flags["remaining"] -= 1
                flags["first"] = False

        def make_tail_norm(ib, o, w, av, h):
            def tail():
                with nc.allow_low_precision(reason="1/den broadcast via f32r matmul"):
                    nc.vector.reciprocal(den_pad[h][64:65, :w], av[h][64:65, :w])
                dbc = pav.tile([128, 512], f32, tag="avy", name=f"dbc_{ib}_{h}")
                nc.tensor.matmul(
                    dbc[:, :w], e64[:], den_pad[h][:, :w], start=True, stop=True
                )
                rb = work.tile([128, 512], f32, tag="rb", name=f"rb_{ib}_{h}")
                nc.vector.tensor_copy(rb[:, :w], dbc[:, :w])
                nc.vector.tensor_tensor(
                    on[0:64, h, o : o + w], av[h][0:64, :w], rb[0:64, :w], OP.mult
                )
            return tail

        def make_tail_proj(ib, o, w):
            def tail():
                ysb = ywork.tile([128, KT, 512], f32, tag="y", name=f"ysb_{ib}")
                for ot in range(KT):
                    py = pav.tile([128, 512], f32, tag="avy", name=f"py_{ib}_{ot}")
                    nc.tensor.matmul(
                        py[:, :w], wp_s[:, :, ot * 128 : (ot + 1) * 128],
                        on[:, :, o : o + w],
                        start=True, stop=True, perf_mode=DRM,
                    )
                    nc.vector.tensor_tensor(
                        ysb[:, ot, :w], py[:, :w], x4s[:, ot, o : o + w], OP.add
                    )
                nc.sync.dma_start(
                    y_d.rearrange("(a p) n -> p a n", p=128)[:, :, o : o + w],
                    ysb[:, :, :w],
                )
            return tail

        # ---------- attention ----------
        deferred = []
        for ib, (o, w) in enumerate(NBLKS):
            ensure_block(ib)
            plan = wave_plan_512() if w == 512 else wave_plan_256()
            av = [None, None]
            avflags = [{"first": True, "remaining": JT}, {"first": True, "remaining": JT}]
            pending = None
            prev_head = None
            for wv_i, (gtag, size, head, jts) in enumerate(plan):
                if av[head] is None:
                    av[head] = pav.tile([65, 512], f32, tag="avy",
                                        name=f"av{head}_{ib}")
                g = pg.tile([128, size, w], f32, tag=gtag, name=f"g_{ib}_{wv_i}")
                emit_sims(g, o, w, head, jts)
                p_sb = pwav.tile([128, size, w], f8, tag="P", name=f"p_{ib}_{wv_i}")
                nc.scalar.activation(p_sb[:], g[:], AF.Exp, bias=bias_m2[:])
                if deferred and wv_i == 0:
                    deferred.pop(0)()
                waves = [pending, (p_sb, head, jts)] if pending else [(p_sb, head, jts)]
                if wv_i < len(plan) - 1:
                    pending = waves.pop()
                for psb_j, head_j, jts_j in waves:
                    for jt in jts_j:
                        ensure_vt(jt)
                    for _ in range(2):
                        if jobs:
                            pop_job()
                    emit_avs(psb_j, w, head_j, jts_j, av[head_j], avflags[head_j])
            make_tail_norm(ib, o, w, av, 0)()
            make_tail_norm(ib, o, w, av, 1)()
            deferred = [make_tail_proj(ib, o, w)]
        while jobs:
            pop_job()
        for fn in deferred:
            fn()

        if debug:
            nc.sync.dma_start(dbg["q2"], q2[:].bitcast(f32))
            nc.sync.dma_start(dbg["k2"], k2[:].bitcast(f32))
            vtf = big.tile([128, JT, 144], f32)
            nc.vector.tensor_copy(vtf[:], vT[:])
            nc.sync.dma_start(dbg["vT"], vtf[:])
            onf = big.tile([65, 2, N], f32)
            nc.vector.tensor_copy(onf[:], on[:])
            nc.sync.dma_start(dbg["on"], onf[:])
            nc.sync.dma_start(dbg["s_bc"], s_bc[:])
            nc.sync.dma_start(dbg["s_colT"], s_colT[:])
        for pool in (ywork, pwav, pav, pg, rsq, work, big):
            pool.release()

    nc.compile()
    return nc


def _get_program():
    if "nc" not in _CACHE:
        _CACHE["nc"] = _build_program()
    return _CACHE["nc"]


def make_in_maps(x, g, w_qkv, w_out, b_out):
    """Build the per-core input dicts for the SPMD launch."""
    import ml_dtypes

    x = np.asarray(x, dtype=np.float32)
    g = np.asarray(g, dtype=np.float32).reshape(DIM)
    w_qkv = np.asarray(w_qkv, dtype=np.float32)
    w_out = np.asarray(w_out, dtype=np.float32)
    b_out = np.asarray(b_out, dtype=np.float32)

    in_maps = []
    for c in range(8):
        beta = c // 4
        h0 = 2 * (c % 4)
        h1 = h0 + 1
        x4 = (x[beta].reshape(DIM, N) / 4.0).astype(np.float32)
        # w_qkv rows: q block [0:512], k block [512:1024], v block [1024:1536]
        qr = np.r_[h0 * DH : (h0 + 1) * DH, h1 * DH : (h1 + 1) * DH]
        wq = w_qkv[qr]            # [128, DIM]
        wk = w_qkv[DIM + qr]      # [128, DIM]
        wvv = w_qkv[2 * DIM + qr]  # [128, DIM]
        gw = (g[None, :] * 4.0).astype(np.float32)
        # fold the attention 1/8 scale into wq so q2 and k2 share s_bc
        wqk = np.concatenate([wq * gw / 8.0, wk * gw], axis=0).T.copy()  # [DIM, 256]
        wvt = (wvv * gw).T.astype(ml_dtypes.bfloat16)  # [DIM, 128]
        wp = np.zeros((65, 2, DIM), dtype=np.float32)
        wp[0:64, 0, :] = w_out[:, h0 * DH : (h0 + 1) * DH].T
        wp[0:64, 1, :] = w_out[:, h1 * DH : (h1 + 1) * DH].T
        wp[64, :, :] = b_out[None, :] / 8.0
        in_maps.append(
            {
                "x4": np.ascontiguousarray(x4),
                "xbin": np.ascontiguousarray(x4.astype(ml_dtypes.bfloat16)),
                "wqk": np.ascontiguousarray(wqk),
                "wv": np.ascontiguousarray(wvt),
                "wp": wp.astype(ml_dtypes.float8_e4m3),
            }
        )
    return in_maps


def run_spmd(in_maps, trace=False):
    from concourse.bass_utils import run_bass_kernel_spmd

    nc = _get_program()
    return run_bass_kernel_spmd(nc, in_maps, list(range(8)), trace=trace)


def combine(results, x):
    x = np.asarray(x, dtype=np.float32)
    y = np.zeros((B, DIM, N), dtype=np.float32)
    for c in range(8):
        y[c // 4] += results[c]["y"]
    return y.reshape(B, DIM, HWS, HWS)


def kernel(x, g, w_qkv, w_out, b_out):
    in_maps = make_in_maps(x, g, w_qkv, w_out, b_out)
    res = run_spmd(in_maps)
    return combine(res.results, x)


# revision 30
# speedup vs baseline: 1.0135x; 1.0135x over previous
"""Trainium2 Bass kernel for nn_Attention_62861141344964.

Full-input contract: kernel(**inputs) takes the unsharded inputs and returns
the full-shape output. Internally shards across 8 NeuronCores as
(batch, head-pair): core c handles batch c//4 and heads {2*(c%4), 2*(c%4)+1}.

Per-core pipeline (ACT-exp is the bottleneck engine; everything else is
arranged around keeping its exp stream dense):
  - prologue per n-block: x DMA -> xsq(fp8, DVE) -> sumsq (fp8 DoubleRow
    matmul) -> DMA round-trip to transposed layout -> rsqrt via DVE bit-trick
    + 2 Newton steps (no ACT sqrt, so ACT runs exp only, one table load) ->
    broadcast (Pool) -> q/k projections (f32r) -> q2/k2 scaled (DVE).
  - attention: sim matmuls f32r -> exp on ACT with bias -2 emitting fp8e4
    directly -> AV as fp8 DoubleRow over j-tile pairs (0.5 cyc/row, two
    j-tiles per instruction) with plain-fp8 orphans; denominator rides row 0
    of vT/av (ones trick).
  - tail per i-block: reciprocal(den) -> Pool partition_broadcast -> DVE
    normalize into `on` (fp8) -> output projection as one fp8-DoubleRow
    matmul per 128-chunk (both heads contracted together) -> residual add on
    Pool -> DMA out.
The host folds g*sqrt(c) (and q's 1/8) into the weights and sums the 4
partial outputs per batch.
"""

import sys

sys.path.insert(0, "/opt/trn_rl_repo")

import numpy as np

HEADS = 8
DH = 64
DIM = 512
B = 2
HWS = 48
N = HWS * HWS  # 2304
KT = 4  # k-tiles of 128 over DIM
JT = 18  # j-tiles of 128 over N
NBLKS = [(0, 512), (512, 512), (1024, 512), (1536, 512), (2048, 256)]
MAGIC_SQRT = 0x1FBD1DF5
SQRT32 = 5.656854249492381

_CACHE = {}


def _build_program(debug=False):
    import concourse.bass as bass  # noqa: F401
    import concourse.mybir as mybir
    import concourse.tile as tile
    from concourse import bacc

    f32 = mybir.dt.float32
    f32r = mybir.dt.float32r
    bf16 = mybir.dt.bfloat16
    f8 = mybir.dt.float8e4
    i32 = mybir.dt.int32
    AF = mybir.ActivationFunctionType
    OP = mybir.AluOpType
    DRM = mybir.MatmulPerfMode.DoubleRow

    nc = bacc.Bacc("TRN2", target_bir_lowering=False, debug=False, num_devices=8)

    x4_d = nc.dram_tensor("x4", [DIM, N], f32r, kind="ExternalInput").ap()
    xb_d = nc.dram_tensor("xbin", [DIM, N], bf16, kind="ExternalInput").ap()
    wqk_d = nc.dram_tensor("wqk", [DIM, 256], f32r, kind="ExternalInput").ap()
    wv_d = nc.dram_tensor("wv", [DIM, 128], bf16, kind="ExternalInput").ap()
    wp_d = nc.dram_tensor("wp", [65, 2, DIM], f8, kind="ExternalInput").ap()
    y_d = nc.dram_tensor("y", [DIM, N], f32, kind="ExternalOutput").ap()
    dbg = {}
    if debug:
        for nm, shp, dt in [("q2", [128, N], f32), ("k2", [128, N], f32),
                            ("vT", [128, JT, 144], f32), ("on", [65, 2, N], f32),
                            ("s_bc", [128, N], f32), ("s_colT", [128, JT], f32)]:
            dbg[nm] = nc.dram_tensor("dbg_" + nm, shp, dt, kind="ExternalOutput").ap()

    with tile.TileContext(nc) as tc:
        big = tc.alloc_tile_pool(name="big", bufs=1)
        work = tc.alloc_tile_pool(name="work", bufs=2)
        rsq = tc.alloc_tile_pool(name="rsq", bufs=6)
        pg = tc.alloc_tile_pool(name="pg", bufs=1, space="PSUM")
        pav = tc.alloc_tile_pool(name="pav", bufs=3, space="PSUM")

        # ---------- persistent tiles ----------
        x4s = big.tile([128, KT, N], f32r)
        xb = big.tile([128, KT, N], bf16)
        q2 = big.tile([128, N], f32r)
        k2 = big.tile([128, N], f32r)
        s_bc = big.tile([128, N], f32)
        s_row = big.tile([1, N], f32)
        t_colT = big.tile([128, JT], f32)
        s_colT = big.tile([128, JT], f32)
        vT = big.tile([128, JT, 144], f8)
        on = big.tile([65, 2, N], f8)
        wqk_s = big.tile([128, KT, 256], f32r)
        wv_s = big.tile([128, KT, 128], bf16)
        wp_s = big.tile([65, 2, DIM], f8)
        ones8 = big.tile([128, 2, 16], f8)
        bias_m2 = big.tile([128, 1], f32)
        e64 = big.tile([128, 128], f32r)  # row 64 = ones: PE partition-bcast of row 64
        den_pad = [big.tile([128, 512], f32r, name="den_pad0"),
                   big.tile([128, 512], f32r, name="den_pad1")]


        nc.gpsimd.memset(ones8[:], 1.0)
        nc.vector.memset(bias_m2[:], -2.0)
        nc.gpsimd.memset(vT[:, :, 64:65], 1.0)
        nc.gpsimd.memset(vT[:, :, 136:137], 1.0)
        nc.vector.memset(e64[:].bitcast(f32), 0.0)
        nc.vector.memset(e64[64:65, :].bitcast(f32), 1.0)

        # x loads split across DMA rings so no single queue serializes them;
        # x block 1 leads the ACT ring (ahead of weights + the bf16 x copy),
        # x block 3 is issued on the Pool ring after the eager block-0
        # prologue so its trigger doesn't block the s broadcast
        x4_r = x4_d.rearrange("(a p) n -> p a n", p=128)
        xb_r = xb_d.rearrange("(a p) n -> p a n", p=128)
        def x_load(bi, eng):
            o, w = NBLKS[bi]
            eng.dma_start(x4s[:, :, o : o + w], x4_r[:, :, o : o + w])
        x_load(0, nc.sync)
        x_load(1, nc.scalar)
        x_load(3, nc.gpsimd)
        x_load(2, nc.sync)
        x_load(4, nc.sync)
        nc.scalar.dma_start(wqk_s[:], wqk_d.rearrange("(a p) m -> p a m", p=128))
        nc.scalar.dma_start(wv_s[:], wv_d.rearrange("(a p) m -> p a m", p=128))
        nc.scalar.dma_start(wp_s[:], wp_d)
        for (o, w) in NBLKS:
            nc.scalar.dma_start(xb[:, :, o : o + w], xb_r[:, :, o : o + w])

        # ---------- prologue jobs (per n-block) ----------
        # The per-token norm scale s = sqrt(32/sumsq) is needed in TWO
        # layouts: as a row (-> partition_broadcast -> s_bc, scaling q2/k2
        # along the free axis) and transposed (s_colT, per-partition scalar
        # for the vT scaling). Computing sumsq in both orientations on the
        # PE (ones-vector matmuls) and running the rsqrt bit-trick + Newton
        # on each avoids any DMA transpose round-trip.
        def pro_sumsq(bi):
            o, w = NBLKS[bi]
            c0, cw = o // 128, w // 128

            def job():
                xsq = work.tile([128, KT, 512], f8, tag="xsq", name=f"xsq_{bi}")
                ps = pav.tile([1, 512], f32, tag="avy", name=f"ps_{bi}")
                for k in range(2):
                    nc.vector.tensor_tensor(
                        xsq[:, 2 * k : 2 * k + 2, :w],
                        x4s[:, 2 * k : 2 * k + 2, o : o + w],
                        x4s[:, 2 * k : 2 * k + 2, o : o + w], OP.mult,
                    )
                    nc.tensor.matmul(
                        ps[:, :w], ones8[:, :, 0:1], xsq[:, 2 * k : 2 * k + 2, :w],
                        start=(k == 0), stop=(k == 1), perf_mode=DRM,
                    )
                pt = pav.tile([128, 512], f32, tag="avy", name=f"pt_{bi}")
                for tt in range(cw):
                    for k in range(2):
                        nc.tensor.matmul(
                            pt[:, tt : tt + 1],
                            xsq[:, 2 * k : 2 * k + 2, tt * 128 : (tt + 1) * 128],
                            ones8[:, :, 0:1],
                            start=(tt == 0 and k == 0),
                            stop=(tt == cw - 1 and k == 1),
                            perf_mode=DRM,
                            skip_group_check=True,
                        )
                # row rsqrt: s_row = sqrt(32/t), seed from bits of 1/t,
                # one Newton step (0.2% worst case). t is copied to SBUF
                # first so the ps PSUM bank frees immediately.
                r0 = work.tile([1, 512], f32, tag="r0", name=f"r0_{bi}")
                r1 = work.tile([1, 512], f32, tag="r1", name=f"r1_{bi}")
                r2 = work.tile([1, 512], f32, tag="r2", name=f"r2_{bi}")
                t_sb = work.tile([1, 512], f32, tag="tsb", name=f"tsb_{bi}")
                nc.vector.tensor_copy(t_sb[:, :w], ps[:, :w])
                nc.vector.reciprocal(r0[:, :w], t_sb[:, :w])
                nc.vector.tensor_scalar(
                    r0[:, :w].bitcast(i32), r0[:, :w].bitcast(i32), 1, None,
                    OP.logical_shift_right,
                )
                nc.vector.tensor_scalar(
                    r0[:, :w].bitcast(i32), r0[:, :w].bitcast(i32), MAGIC_SQRT,
                    None, OP.add,
                )
                nc.vector.tensor_tensor(r1[:, :w], r0[:, :w], r0[:, :w], OP.mult)
                nc.vector.tensor_tensor(r2[:, :w], r1[:, :w], t_sb[:, :w], OP.mult)
                nc.vector.tensor_scalar(
                    r2[:, :w], r2[:, :w], -0.5 * SQRT32, 1.5 * SQRT32, OP.mult, OP.add
                )
                nc.vector.tensor_tensor(s_row[:, o : o + w], r2[:, :w], r0[:, :w], OP.mult)
                nc.gpsimd.partition_broadcast(s_bc[:, o : o + w], s_row[:, o : o + w])
                # transposed rsqrt for s_colT (tiny frees; two Newton steps)
                tc_ = rsq.tile([128, 4], f32, tag="rsq", name=f"tc_{bi}")
                nc.vector.tensor_copy(tc_[:, :cw], pt[:, :cw])
                y0 = rsq.tile([128, 4], f32, tag="rsq", name=f"y0_{bi}")
                aa = rsq.tile([128, 4], f32, tag="rsq", name=f"aa_{bi}")
                bb = rsq.tile([128, 4], f32, tag="rsq", name=f"bb_{bi}")
                nc.vector.reciprocal(y0[:, :cw], tc_[:, :cw])
                nc.vector.tensor_scalar(
                    y0[:, :cw].bitcast(i32), y0[:, :cw].bitcast(i32), 1, None,
                    OP.logical_shift_right,
                )
                nc.vector.tensor_scalar(
                    y0[:, :cw].bitcast(i32), y0[:, :cw].bitcast(i32), MAGIC_SQRT,
                    None, OP.add,
                )
                nc.vector.tensor_tensor(aa[:, :cw], y0[:, :cw], y0[:, :cw], OP.mult)
                nc.vector.tensor_tensor(bb[:, :cw], aa[:, :cw], tc_[:, :cw], OP.mult)
                nc.vector.tensor_scalar(
                    aa[:, :cw], bb[:, :cw], -0.5 * SQRT32, 1.5 * SQRT32, OP.mult, OP.add
                )
                nc.vector.tensor_tensor(
                    s_colT[:, c0 : c0 + cw], aa[:, :cw], y0[:, :cw], OP.mult
                )
            return job

        def pro_k(bi):
            o, w = NBLKS[bi]

            def job():
                pk = pav.tile([128, 512], f32, tag="avy", name=f"pk_{bi}")
                for kt in range(KT):
                    nc.tensor.matmul(
                        pk[:, :w], wqk_s[:, kt, 128:256], x4s[:, kt, o : o + w],
                        start=(kt == 0), stop=(kt == KT - 1),
                    )
                nc.vector.tensor_tensor(
                    k2[:, o : o + w], pk[:, :w], s_bc[:, o : o + w], OP.mult
                )
            return job

        def pro_q(bi):
            o, w = NBLKS[bi]

            def job():
                pq = pav.tile([128, 512], f32, tag="avy", name=f"pq_{bi}")
                for kt in range(KT):
                    nc.tensor.matmul(
                        pq[:, :w], wqk_s[:, kt, 0:128], x4s[:, kt, o : o + w],
                        start=(kt == 0), stop=(kt == KT - 1),
                    )
                nc.vector.tensor_tensor(
                    q2[:, o : o + w], pq[:, :w], s_bc[:, o : o + w], OP.mult
                )
            return job

        def vt_job(jt):
            def job():
                pv = pav.tile([128, 512], f32, tag="avy", name=f"pv_{jt}")
                for kt in range(KT):
                    nc.tensor.matmul(
                        pv[:, :128], xb[:, kt, jt * 128 : (jt + 1) * 128],
                        wv_s[:, kt, :], start=(kt == 0), stop=(kt == KT - 1),
                    )
                nc.vector.tensor_scalar_mul(
                    vT[:, jt, 0:64], pv[:, 0:64], s_colT[:, jt : jt + 1]
                )
                nc.vector.tensor_scalar_mul(
                    vT[:, jt, 72:136], pv[:, 64:128], s_colT[:, jt : jt + 1]
                )
            return job

        # job queue: prologue for blocks 1.. interleaved with vT jobs, popped
        # during the attention waves (block 0's prologue is emitted eagerly)
        jobs = []
        pro_done = [False] * len(NBLKS)
        vt_done = [False] * JT

        def mark_pro(bi):
            def f():
                pro_done[bi] = True
            return f

        def mark_vt(jt):
            def f():
                vt_done[jt] = True
            return f

        for fn in (pro_sumsq(0), pro_k(0), pro_q(0)):
            fn()
        pro_done[0] = True

        def late_memsets():
            nc.gpsimd.memset(on[64:65, :, :], 1.0)
            nc.gpsimd.memset(den_pad[0][:].bitcast(f32), 0.0)
            nc.gpsimd.memset(den_pad[1][:].bitcast(f32), 0.0)

        for bi in range(1, len(NBLKS)):
            jobs += [pro_sumsq(bi), pro_k(bi),
                     (pro_q(bi), mark_pro(bi))]
            lo = 4 * (bi - 1)
            jobs += [(vt_job(jt), mark_vt(jt)) for jt in range(lo, min(lo + 4, JT))]
        jobs.append(late_memsets)
        jobs += [(vt_job(jt), mark_vt(jt)) for jt in range(16, JT)]

        def pop_job():
            j = jobs.pop(0)
            if isinstance(j, tuple):
                j[0]()
                j[1]()
            else:
                j()

        def ensure_vt(jt):
            while not vt_done[jt]:
                pop_job()

        def ensure_block(bi):
            while not pro_done[bi]:
                pop_job()

        pwav = tc.alloc_tile_pool(name="pwav", bufs=10)
        ywork = tc.alloc_tile_pool(name="ywork", bufs=3)

        # ---------- wave plans ----------
        # full blocks (w=512): alternating G3/G2 tiles of (head, [jts]);
        # 3-waves carry a DR pair + an orphan, 2-waves a DR pair.
        def wave_plan_512():
            plan = []  # (gtag, size, head, jts)
            a = list(range(JT))
            b = list(range(JT))
            for i in range(14):
                size = [3, 3, 2, 3, 2, 3, 2, 3, 2, 3, 2, 3, 2, 3][i]
                head, src = (0, a) if (i % 2 == 0 or i == 13) else (1, b)
                jts = [src.pop(0) for _ in range(size)]
                plan.append(("G3" if size == 3 else "G2", size, head, jts))
            assert not a and not b, (a, b)
            return plan

        def wave_plan_256():
            plan = []
            a = list(range(JT))
            b = list(range(JT))
            for i in range(7):
                size = [6, 4, 6, 4, 6, 4, 6][i]
                head, src = (0, a) if (i % 2 == 0 and i < 6) else (1, b)
                jts = [src.pop(0) for _ in range(size)]
                plan.append(("G3" if size == 6 else "G2", size, head, jts))
            assert not a and not b, (a, b)
            return plan

        def emit_sims(g, o, w, head, jts):
            for slot, jt in enumerate(jts):
                nc.tensor.matmul(
                    g[:, slot, :],
                    k2[64 * head : 64 * (head + 1), jt * 128 : (jt + 1) * 128],
                    q2[64 * head : 64 * (head + 1), o : o + w],
                    start=True, stop=True,
                )

        def emit_avs(psb, w, head, jts, av, flags):
            # DR pairs over consecutive slots, plain fp8 for the odd orphan
            vbase = 72 * head
            i = 0
            while i < len(jts):
                first = flags["first"]
                if i + 1 < len(jts) and jts[i + 1] == jts[i] + 1:
                    last = flags["remaining"] == 2
                    nc.tensor.matmul(
                        av[:, :w],
                        vT[:, jts[i] : jts[i] + 2, vbase : vbase + 65],
                        psb[:, i : i + 2, :],
                        start=first, stop=last, perf_mode=DRM,
                        skip_group_check=True,
                    )
                    i += 2
                    flags["remaining"] -= 2
                else:
                    last = flags["remaining"] == 1
                    nc.tensor.matmul(
                        av[:, :w],
                        vT[:, jts[i], vbase : vbase + 65],
                        psb[:, i, :],
                        start=first, stop=last, skip_group_check=True,
                    )
                    i += 1
                    flags["remaining"] -= 1
                flags["first"] = False

        def make_tail_norm(ib, o, w, av, h):
            def tail():
                with nc.allow_low_precision(reason="1/den broadcast via f32r matmul"):
                    nc.vector.reciprocal(den_pad[h][64:65, :w], av[h][64:65, :w])
                dbc = pav.tile([128, 512], f32, tag="avy", name=f"dbc_{ib}_{h}")
                nc.tensor.matmul(
                    dbc[:, :w], e64[:], den_pad[h][:, :w], start=True, stop=True
                )
                rb = work.tile([128, 512], f32, tag="rb", name=f"rb_{ib}_{h}")
                nc.vector.tensor_copy(rb[:, :w], dbc[:, :w])
                nc.vector.tensor_tensor(
                    on[0:64, h, o : o + w], av[h][0:64, :w], rb[0:64, :w], OP.mult
                )
            return tail

        def make_tail_proj(ib, o, w):
            def tail():
                ysb = ywork.tile([128, KT, 512], f32, tag="y", name=f"ysb_{ib}")
                for ot in range(KT):
                    py = pav.tile([128, 512], f32, tag="avy", name=f"py_{ib}_{ot}")
                    nc.tensor.matmul(
                        py[:, :w], wp_s[:, :, ot * 128 : (ot + 1) * 128],
                        on[:, :, o : o + w],
                        start=True, stop=True, perf_mode=DRM,
                    )
                    nc.vector.tensor_tensor(
                        ysb[:, ot, :w], py[:, :w], x4s[:, ot, o : o + w], OP.add
                    )
                nc.sync.dma_start(
                    y_d.rearrange("(a p) n -> p a n", p=128)[:, :, o : o + w],
                    ysb[:, :, :w],
                )
            return tail

        # ---------- attention ----------
        deferred = []
        for ib, (o, w) in enumerate(NBLKS):
            ensure_block(ib)
            plan = wave_plan_512() if w == 512 else wave_plan_256()
            av = [None, None]
            avflags = [{"first": True, "remaining": JT}, {"first": True, "remaining": JT}]
            pending = None
            prev_head = None
            for wv_i, (gtag, size, head, jts) in enumerate(plan):
                if av[head] is None:
                    av[head] = pav.tile([65, 512], f32, tag="avy",
                                        name=f"av{head}_{ib}")
                g = pg.tile([128, size, w], f32, tag=gtag, name=f"g_{ib}_{wv_i}")
                emit_sims(g, o, w, head, jts)
                p_sb = pwav.tile([128, size, w], f8, tag="P", name=f"p_{ib}_{wv_i}")
                nc.scalar.activation(p_sb[:], g[:], AF.Exp, bias=bias_m2[:])
                if deferred and wv_i == 0:
                    deferred.pop(0)()
                waves = [pending, (p_sb, head, jts)] if pending else [(p_sb, head, jts)]
                if wv_i < len(plan) - 1:
                    pending = waves.pop()
                for psb_j, head_j, jts_j in waves:
                    for jt in jts_j:
                        ensure_vt(jt)
                    for _ in range(2):
                        if jobs:
                            pop_job()
                    emit_avs(psb_j, w, head_j, jts_j, av[head_j], avflags[head_j])
            make_tail_norm(ib, o, w, av, 0)()
            make_tail_norm(ib, o, w, av, 1)()
            deferred = [make_tail_proj(ib, o, w)]
        while jobs:
            pop_job()
        for fn in deferred:
            fn()

        if debug:
            nc.sync.dma_start(dbg["q2"], q2[:].bitcast(f32))
            nc.sync.dma_start(dbg["k2"], k2[:].bitcast(f32))
            vtf = big.tile([128, JT, 144], f32)
            nc.vector.tensor_copy(vtf[:], vT[:])
            nc.sync.dma_start(dbg["vT"], vtf[:])
            onf = big.tile([65, 2, N], f32)
            nc.vector.tensor_copy(onf[:], on[:])
            nc.sync.dma_start(dbg["on"], onf[:])
            nc.sync.dma_start(dbg["s_bc"], s_bc[:])
            nc.sync.dma_start(dbg["s_colT"], s_colT[:])
        for pool in (ywork, pwav, pav, pg, rsq, work, big):
            pool.release()

    nc.compile()
    return nc


def _get_program():
    if "nc" not in _CACHE:
        _CACHE["nc"] = _build_program()
    return _CACHE["nc"]


def make_in_maps(x, g, w_qkv, w_out, b_out):
    """Build the per-core input dicts for the SPMD launch."""
    import ml_dtypes

    x = np.asarray(x, dtype=np.float32)
    g = np.asarray(g, dtype=np.float32).reshape(DIM)
    w_qkv = np.asarray(w_qkv, dtype=np.float32)
    w_out = np.asarray(w_out, dtype=np.float32)
    b_out = np.asarray(b_out, dtype=np.float32)

    in_maps = []
    for c in range(8):
        beta = c // 4
        h0 = 2 * (c % 4)
        h1 = h0 + 1
        x4 = (x[beta].reshape(DIM, N) / 4.0).astype(np.float32)
        # w_qkv rows: q block [0:512], k block [512:1024], v block [1024:1536]
        qr = np.r_[h0 * DH : (h0 + 1) * DH, h1 * DH : (h1 + 1) * DH]
        wq = w_qkv[qr]            # [128, DIM]
        wk = w_qkv[DIM + qr]      # [128, DIM]
        wvv = w_qkv[2 * DIM + qr]  # [128, DIM]
        gw = (g[None, :] * 4.0).astype(np.float32)
        # fold the attention 1/8 scale into wq so q2 and k2 share s_bc
        wqk = np.concatenate([wq * gw / 8.0, wk * gw], axis=0).T.copy()  # [DIM, 256]
        wvt = (wvv * gw).T.astype(ml_dtypes.bfloat16)  # [DIM, 128]
        wp = np.zeros((65, 2, DIM), dtype=np.float32)
        wp[0:64, 0, :] = w_out[:, h0 * DH : (h0 + 1) * DH].T
        wp[0:64, 1, :] = w_out[:, h1 * DH : (h1 + 1) * DH].T
        wp[64, :, :] = b_out[None, :] / 8.0
        in_maps.append(
            {
                "x4": np.ascontiguousarray(x4),
                "xbin": np.ascontiguousarray(x4.astype(ml_dtypes.bfloat16)),
                "wqk": np.ascontiguousarray(wqk),
                "wv": np.ascontiguousarray(wvt),
                "wp": wp.astype(ml_dtypes.float8_e4m3),
            }
        )
    return in_maps


def run_spmd(in_maps, trace=False):
    from concourse.bass_utils import run_bass_kernel_spmd

    nc = _get_program()
    return run_bass_kernel_spmd(nc, in_maps, list(range(8)), trace=trace)


def combine(results, x):
    x = np.asarray(x, dtype=np.float32)
    y = np.zeros((B, DIM, N), dtype=np.float32)
    for c in range(8):
        y[c // 4] += results[c]["y"]
    return y.reshape(B, DIM, HWS, HWS)


def kernel(x, g, w_qkv, w_out, b_out):
    in_maps = make_in_maps(x, g, w_qkv, w_out, b_out)
    res = run_spmd(in_maps)
    return combine(res.results, x)


# revision 31
# speedup vs baseline: 1.0878x; 1.0733x over previous
"""Trainium2 Bass kernel for nn_Attention_62861141344964.

Full-input contract: kernel(**inputs) takes the unsharded inputs and returns
the full-shape output. Internally shards across 8 NeuronCores as
(batch, head-pair): core c handles batch c//4 and heads {2*(c%4), 2*(c%4)+1}.

Per-core pipeline (ACT-exp is the bottleneck engine; everything else is
arranged around keeping its exp stream dense):
  - prologue per n-block: x DMA -> xsq(fp8, DVE) -> sumsq (fp8 DoubleRow
    matmul) -> DMA round-trip to transposed layout -> rsqrt via DVE bit-trick
    + 2 Newton steps (no ACT sqrt, so ACT runs exp only, one table load) ->
    broadcast (Pool) -> q/k projections (f32r) -> q2/k2 scaled (DVE).
  - attention: sim matmuls f32r -> exp on ACT with bias -2 emitting fp8e4
    directly -> AV as fp8 DoubleRow over j-tile pairs (0.5 cyc/row, two
    j-tiles per instruction) with plain-fp8 orphans; denominator rides row 0
    of vT/av (ones trick).
  - tail per i-block: reciprocal(den) -> Pool partition_broadcast -> DVE
    normalize into `on` (fp8) -> output projection as one fp8-DoubleRow
    matmul per 128-chunk (both heads contracted together) -> residual add on
    Pool -> DMA out.
The host folds g*sqrt(c) (and q's 1/8) into the weights and sums the 4
partial outputs per batch.
"""

import sys

sys.path.insert(0, "/opt/trn_rl_repo")

import numpy as np

HEADS = 8
DH = 64
DIM = 512
B = 2
HWS = 48
N = HWS * HWS  # 2304
KT = 4  # k-tiles of 128 over DIM
JT = 18  # j-tiles of 128 over N
NBLKS = [(0, 512), (512, 512), (1024, 512), (1536, 512), (2048, 256)]
MAGIC_SQRT = 0x1FBD1DF5
SQRT32 = 5.656854249492381

_CACHE = {}


def _build_program(debug=False):
    import concourse.bass as bass  # noqa: F401
    import concourse.mybir as mybir
    import concourse.tile as tile
    from concourse import bacc

    f32 = mybir.dt.float32
    f32r = mybir.dt.float32r
    bf16 = mybir.dt.bfloat16
    f8 = mybir.dt.float8e4
    i32 = mybir.dt.int32
    AF = mybir.ActivationFunctionType
    OP = mybir.AluOpType
    DRM = mybir.MatmulPerfMode.DoubleRow

    nc = bacc.Bacc("TRN2", target_bir_lowering=False, debug=False, num_devices=8)

    xb_d = nc.dram_tensor("xbin", [DIM, N], bf16, kind="ExternalInput").ap()
    wqk_d = nc.dram_tensor("wqk", [DIM, 256], bf16, kind="ExternalInput").ap()
    wv_d = nc.dram_tensor("wv", [DIM, 128], bf16, kind="ExternalInput").ap()
    wp_d = nc.dram_tensor("wp", [65, 2, DIM], f8, kind="ExternalInput").ap()
    y_d = nc.dram_tensor("y", [DIM, N], f32, kind="ExternalOutput").ap()
    dbg = {}
    if debug:
        for nm, shp, dt in [("q2", [128, N], f32), ("k2", [128, N], f32),
                            ("vT", [128, JT, 144], f32), ("on", [65, 2, N], f32),
                            ("s_bc", [128, N], f32), ("s_colT", [128, JT], f32)]:
            dbg[nm] = nc.dram_tensor("dbg_" + nm, shp, dt, kind="ExternalOutput").ap()

    with tile.TileContext(nc) as tc:
        big = tc.alloc_tile_pool(name="big", bufs=1)
        work = tc.alloc_tile_pool(name="work", bufs=2)
        rsq = tc.alloc_tile_pool(name="rsq", bufs=6)
        pg = tc.alloc_tile_pool(name="pg", bufs=1, space="PSUM")
        pav = tc.alloc_tile_pool(name="pav", bufs=3, space="PSUM")

        # ---------- persistent tiles ----------
        xb = big.tile([128, KT, N], bf16)
        q2 = big.tile([128, N], f32r)
        k2 = big.tile([128, N], f32r)
        s_bc = big.tile([128, N], f32)
        s_row = big.tile([1, N], f32)
        t_colT = big.tile([128, JT], f32)
        s_colT = big.tile([128, JT], f32)
        vT = big.tile([128, JT, 144], f8)
        on = big.tile([65, 2, N], f8)
        wqk_s = big.tile([128, KT, 256], bf16)
        wv_s = big.tile([128, KT, 128], bf16)
        wp_s = big.tile([65, 2, DIM], f8)
        ones8 = big.tile([128, 2, 16], f8)
        bias_m2 = big.tile([128, 1], f32)
        e64 = big.tile([128, 128], f32r)  # row 64 = ones: PE partition-bcast of row 64
        den_pad = [big.tile([128, 512], f32r, name="den_pad0"),
                   big.tile([128, 512], f32r, name="den_pad1")]


        nc.gpsimd.memset(ones8[:], 1.0)
        nc.vector.memset(bias_m2[:], -2.0)
        nc.gpsimd.memset(vT[:, :, 64:65], 1.0)
        nc.gpsimd.memset(vT[:, :, 136:137], 1.0)
        nc.vector.memset(e64[:].bitcast(f32), 0.0)
        nc.vector.memset(e64[64:65, :].bitcast(f32), 1.0)

        # x is shipped ONLY as bf16 (the cost model's DMA engines serialize
        # all transfers, so halving startup bytes halves the lead-in). wqk
        # leads so the block-0 projections can start immediately.
        xb_r = xb_d.rearrange("(a p) n -> p a n", p=128)
        nc.scalar.dma_start(wqk_s[:], wqk_d.rearrange("(a p) m -> p a m", p=128))
        o, w = NBLKS[0]
        nc.sync.dma_start(xb[:, :, o : o + w], xb_r[:, :, o : o + w])
        nc.scalar.dma_start(wv_s[:], wv_d.rearrange("(a p) m -> p a m", p=128))
        nc.scalar.dma_start(wp_s[:], wp_d)
        for (o, w) in NBLKS[1:]:
            nc.sync.dma_start(xb[:, :, o : o + w], xb_r[:, :, o : o + w])

        # ---------- prologue jobs (per n-block) ----------
        # The per-token norm scale s = sqrt(32/sumsq) is needed in TWO
        # layouts: as a row (-> partition_broadcast -> s_bc, scaling q2/k2
        # along the free axis) and transposed (s_colT, per-partition scalar
        # for the vT scaling). Computing sumsq in both orientations on the
        # PE (ones-vector matmuls) and running the rsqrt bit-trick + Newton
        # on each avoids any DMA transpose round-trip.
        def pro_sumsq(bi):
            o, w = NBLKS[bi]
            c0, cw = o // 128, w // 128

            def job():
                xsq = work.tile([128, KT, 512], f8, tag="xsq", name=f"xsq_{bi}")
                ps = pav.tile([1, 512], f32, tag="avy", name=f"ps_{bi}")
                for k in range(2):
                    nc.vector.tensor_tensor(
                        xsq[:, 2 * k : 2 * k + 2, :w],
                        xb[:, 2 * k : 2 * k + 2, o : o + w],
                        xb[:, 2 * k : 2 * k + 2, o : o + w], OP.mult,
                    )
                    nc.tensor.matmul(
                        ps[:, :w], ones8[:, :, 0:1], xsq[:, 2 * k : 2 * k + 2, :w],
                        start=(k == 0), stop=(k == 1), perf_mode=DRM,
                    )
                pt = pav.tile([128, 512], f32, tag="avy", name=f"pt_{bi}")
                for tt in range(cw):
                    for k in range(2):
                        nc.tensor.matmul(
                            pt[:, tt : tt + 1],
                            xsq[:, 2 * k : 2 * k + 2, tt * 128 : (tt + 1) * 128],
                            ones8[:, :, 0:1],
                            start=(tt == 0 and k == 0),
                            stop=(tt == cw - 1 and k == 1),
                            perf_mode=DRM,
                            skip_group_check=True,
                        )
                # row rsqrt: s_row = sqrt(32/t), seed from bits of 1/t,
                # one Newton step (0.2% worst case). t is copied to SBUF
                # first so the ps PSUM bank frees immediately.
                r0 = work.tile([1, 512], f32, tag="r0", name=f"r0_{bi}")
                r1 = work.tile([1, 512], f32, tag="r1", name=f"r1_{bi}")
                r2 = work.tile([1, 512], f32, tag="r2", name=f"r2_{bi}")
                t_sb = work.tile([1, 512], f32, tag="tsb", name=f"tsb_{bi}")
                nc.vector.tensor_copy(t_sb[:, :w], ps[:, :w])
                nc.vector.reciprocal(r0[:, :w], t_sb[:, :w])
                nc.vector.tensor_scalar(
                    r0[:, :w].bitcast(i32), r0[:, :w].bitcast(i32), 1, None,
                    OP.logical_shift_right,
                )
                nc.vector.tensor_scalar(
                    r0[:, :w].bitcast(i32), r0[:, :w].bitcast(i32), MAGIC_SQRT,
                    None, OP.add,
                )
                nc.vector.tensor_tensor(r1[:, :w], r0[:, :w], r0[:, :w], OP.mult)
                nc.vector.tensor_tensor(r2[:, :w], r1[:, :w], t_sb[:, :w], OP.mult)
                nc.vector.tensor_scalar(
                    r2[:, :w], r2[:, :w], -0.5 * SQRT32, 1.5 * SQRT32, OP.mult, OP.add
                )
                nc.vector.tensor_tensor(s_row[:, o : o + w], r2[:, :w], r0[:, :w], OP.mult)
                nc.gpsimd.partition_broadcast(s_bc[:, o : o + w], s_row[:, o : o + w])
                # transposed rsqrt for s_colT (tiny frees; two Newton steps)
                tc_ = rsq.tile([128, 4], f32, tag="rsq", name=f"tc_{bi}")
                nc.vector.tensor_copy(tc_[:, :cw], pt[:, :cw])
                y0 = rsq.tile([128, 4], f32, tag="rsq", name=f"y0_{bi}")
                aa = rsq.tile([128, 4], f32, tag="rsq", name=f"aa_{bi}")
                bb = rsq.tile([128, 4], f32, tag="rsq", name=f"bb_{bi}")
                nc.vector.reciprocal(y0[:, :cw], tc_[:, :cw])
                nc.vector.tensor_scalar(
                    y0[:, :cw].bitcast(i32), y0[:, :cw].bitcast(i32), 1, None,
                    OP.logical_shift_right,
                )
                nc.vector.tensor_scalar(
                    y0[:, :cw].bitcast(i32), y0[:, :cw].bitcast(i32), MAGIC_SQRT,
                    None, OP.add,
                )
                nc.vector.tensor_tensor(aa[:, :cw], y0[:, :cw], y0[:, :cw], OP.mult)
                nc.vector.tensor_tensor(bb[:, :cw], aa[:, :cw], tc_[:, :cw], OP.mult)
                nc.vector.tensor_scalar(
                    aa[:, :cw], bb[:, :cw], -0.5 * SQRT32, 1.5 * SQRT32, OP.mult, OP.add
                )
                nc.vector.tensor_tensor(
                    s_colT[:, c0 : c0 + cw], aa[:, :cw], y0[:, :cw], OP.mult
                )
            return job

        def pro_k(bi):
            o, w = NBLKS[bi]

            def job():
                pk = pav.tile([128, 512], f32, tag="avy", name=f"pk_{bi}")
                for kt in range(KT):
                    nc.tensor.matmul(
                        pk[:, :w], wqk_s[:, kt, 128:256], xb[:, kt, o : o + w],
                        start=(kt == 0), stop=(kt == KT - 1),
                    )
                nc.vector.tensor_tensor(
                    k2[:, o : o + w], pk[:, :w], s_bc[:, o : o + w], OP.mult
                )
            return job

        def pro_q(bi):
            o, w = NBLKS[bi]

            def job():
                pq = pav.tile([128, 512], f32, tag="avy", name=f"pq_{bi}")
                for kt in range(KT):
                    nc.tensor.matmul(
                        pq[:, :w], wqk_s[:, kt, 0:128], xb[:, kt, o : o + w],
                        start=(kt == 0), stop=(kt == KT - 1),
                    )
                nc.vector.tensor_tensor(
                    q2[:, o : o + w], pq[:, :w], s_bc[:, o : o + w], OP.mult
                )
            return job

        def vt_job(jt):
            def job():
                pv = pav.tile([128, 512], f32, tag="avy", name=f"pv_{jt}")
                for kt in range(KT):
                    nc.tensor.matmul(
                        pv[:, :128], xb[:, kt, jt * 128 : (jt + 1) * 128],
                        wv_s[:, kt, :], start=(kt == 0), stop=(kt == KT - 1),
                    )
                nc.vector.tensor_scalar_mul(
                    vT[:, jt, 0:64], pv[:, 0:64], s_colT[:, jt : jt + 1]
                )
                nc.vector.tensor_scalar_mul(
                    vT[:, jt, 72:136], pv[:, 64:128], s_colT[:, jt : jt + 1]
                )
            return job

        # job queue: prologue for blocks 1.. interleaved with vT jobs, popped
        # during the attention waves (block 0's prologue is emitted eagerly)
        jobs = []
        pro_done = [False] * len(NBLKS)
        vt_done = [False] * JT

        def mark_pro(bi):
            def f():
                pro_done[bi] = True
            return f

        def mark_vt(jt):
            def f():
                vt_done[jt] = True
            return f

        for fn in (pro_sumsq(0), pro_k(0), pro_q(0)):
            fn()
        pro_done[0] = True

        def late_memsets():
            nc.gpsimd.memset(on[64:65, :, :], 1.0)
            nc.gpsimd.memset(den_pad[0][:].bitcast(f32), 0.0)
            nc.gpsimd.memset(den_pad[1][:].bitcast(f32), 0.0)

        for bi in range(1, len(NBLKS)):
            jobs += [pro_sumsq(bi), pro_k(bi),
                     (pro_q(bi), mark_pro(bi))]
            lo = 4 * (bi - 1)
            jobs += [(vt_job(jt), mark_vt(jt)) for jt in range(lo, min(lo + 4, JT))]
        jobs.append(late_memsets)
        jobs += [(vt_job(jt), mark_vt(jt)) for jt in range(16, JT)]

        def pop_job():
            j = jobs.pop(0)
            if isinstance(j, tuple):
                j[0]()
                j[1]()
            else:
                j()

        def ensure_vt(jt):
            while not vt_done[jt]:
                pop_job()

        def ensure_block(bi):
            while not pro_done[bi]:
                pop_job()

        pwav = tc.alloc_tile_pool(name="pwav", bufs=10)
        ywork = tc.alloc_tile_pool(name="ywork", bufs=3)

        # ---------- wave plans ----------
        # full blocks (w=512): alternating G3/G2 tiles of (head, [jts]);
        # 3-waves carry a DR pair + an orphan, 2-waves a DR pair.
        def wave_plan_512():
            plan = []  # (gtag, size, head, jts)
            a = list(range(JT))
            b = list(range(JT))
            for i in range(14):
                size = [3, 3, 2, 3, 2, 3, 2, 3, 2, 3, 2, 3, 2, 3][i]
                head, src = (0, a) if (i % 2 == 0 or i == 13) else (1, b)
                jts = [src.pop(0) for _ in range(size)]
                plan.append(("G3" if size == 3 else "G2", size, head, jts))
            assert not a and not b, (a, b)
            return plan

        def wave_plan_256():
            plan = []
            a = list(range(JT))
            b = list(range(JT))
            for i in range(7):
                size = [6, 4, 6, 4, 6, 4, 6][i]
                head, src = (0, a) if (i % 2 == 0 and i < 6) else (1, b)
                jts = [src.pop(0) for _ in range(size)]
                plan.append(("G3" if size == 6 else "G2", size, head, jts))
            assert not a and not b, (a, b)
            return plan

        def emit_sims(g, o, w, head, jts):
            for slot, jt in enumerate(jts):
                nc.tensor.matmul(
                    g[:, slot, :],
                    k2[64 * head : 64 * (head + 1), jt * 128 : (jt + 1) * 128],
                    q2[64 * head : 64 * (head + 1), o : o + w],
                    start=True, stop=True,
                )

        def emit_avs(psb, w, head, jts, av, flags):
            # DR pairs over consecutive slots, plain fp8 for the odd orphan
            vbase = 72 * head
            i = 0
            while i < len(jts):
                first = flags["first"]
                if i + 1 < len(jts) and jts[i + 1] == jts[i] + 1:
                    last = flags["remaining"] == 2
                    nc.tensor.matmul(
                        av[:, :w],
                        vT[:, jts[i] : jts[i] + 2, vbase : vbase + 65],
                        psb[:, i : i + 2, :],
                        start=first, stop=last, perf_mode=DRM,
                        skip_group_check=True,
                    )
                    i += 2
                    flags["remaining"] -= 2
                else:
                    last = flags["remaining"] == 1
                    nc.tensor.matmul(
                        av[:, :w],
                        vT[:, jts[i], vbase : vbase + 65],
                        psb[:, i, :],
                        start=first, stop=last, skip_group_check=True,
                    )
                    i += 1
                    flags["remaining"] -= 1
                flags["first"] = False

        def make_tail_norm(ib, o, w, av, h):
            def tail():
                with nc.allow_low_precision(reason="1/den broadcast via f32r matmul"):
                    nc.vector.reciprocal(den_pad[h][64:65, :w], av[h][64:65, :w])
                dbc = pav.tile([128, 512], f32, tag="avy", name=f"dbc_{ib}_{h}")
                nc.tensor.matmul(
                    dbc[:, :w], e64[:], den_pad[h][:, :w], start=True, stop=True
                )
                rb = work.tile([128, 512], f32, tag="rb", name=f"rb_{ib}_{h}")
                nc.vector.tensor_copy(rb[:, :w], dbc[:, :w])
                nc.vector.tensor_tensor(
                    on[0:64, h, o : o + w], av[h][0:64, :w], rb[0:64, :w], OP.mult
                )
            return tail

        def make_tail_proj(ib, o, w):
            def tail():
                ysb = ywork.tile([128, KT, 512], f32, tag="y", name=f"ysb_{ib}")
                for ot in range(KT):
                    py = pav.tile([128, 512], f32, tag="avy", name=f"py_{ib}_{ot}")
                    nc.tensor.matmul(
                        py[:, :w], wp_s[:, :, ot * 128 : (ot + 1) * 128],
                        on[:, :, o : o + w],
                        start=True, stop=True, perf_mode=DRM,
                    )
                    nc.vector.tensor_tensor(
                        ysb[:, ot, :w], py[:, :w], xb[:, ot, o : o + w], OP.add
                    )
                nc.sync.dma_start(
                    y_d.rearrange("(a p) n -> p a n", p=128)[:, :, o : o + w],
                    ysb[:, :, :w],
                )
            return tail

        # ---------- attention ----------
        deferred = []
        for ib, (o, w) in enumerate(NBLKS):
            ensure_block(ib)
            plan = wave_plan_512() if w == 512 else wave_plan_256()
            av = [None, None]
            avflags = [{"first": True, "remaining": JT}, {"first": True, "remaining": JT}]
            pending = None
            prev_head = None
            for wv_i, (gtag, size, head, jts) in enumerate(plan):
                if av[head] is None:
                    av[head] = pav.tile([65, 512], f32, tag="avy",
                                        name=f"av{head}_{ib}")
                g = pg.tile([128, size, w], f32, tag=gtag, name=f"g_{ib}_{wv_i}")
                emit_sims(g, o, w, head, jts)
                p_sb = pwav.tile([128, size, w], f8, tag="P", name=f"p_{ib}_{wv_i}")
                nc.scalar.activation(p_sb[:], g[:], AF.Exp, bias=bias_m2[:])
                if deferred and wv_i == 0:
                    deferred.pop(0)()
                waves = [pending, (p_sb, head, jts)] if pending else [(p_sb, head, jts)]
                if wv_i < len(plan) - 1:
                    pending = waves.pop()
                for psb_j, head_j, jts_j in waves:
                    for jt in jts_j:
                        ensure_vt(jt)
                    for _ in range(2):
                        if jobs:
                            pop_job()
                    emit_avs(psb_j, w, head_j, jts_j, av[head_j], avflags[head_j])
            make_tail_norm(ib, o, w, av, 0)()
            make_tail_norm(ib, o, w, av, 1)()
            deferred = [make_tail_proj(ib, o, w)]
        while jobs:
            pop_job()
        for fn in deferred:
            fn()

        if debug:
            nc.sync.dma_start(dbg["q2"], q2[:].bitcast(f32))
            nc.sync.dma_start(dbg["k2"], k2[:].bitcast(f32))
            vtf = big.tile([128, JT, 144], f32)
            nc.vector.tensor_copy(vtf[:], vT[:])
            nc.sync.dma_start(dbg["vT"], vtf[:])
            onf = big.tile([65, 2, N], f32)
            nc.vector.tensor_copy(onf[:], on[:])
            nc.sync.dma_start(dbg["on"], onf[:])
            nc.sync.dma_start(dbg["s_bc"], s_bc[:])
            nc.sync.dma_start(dbg["s_colT"], s_colT[:])
        for pool in (ywork, pwav, pav, pg, rsq, work, big):
            pool.release()

    nc.compile()
    return nc


def _get_program():
    if "nc" not in _CACHE:
        _CACHE["nc"] = _build_program()
    return _CACHE["nc"]


def make_in_maps(x, g, w_qkv, w_out, b_out):
    """Build the per-core input dicts for the SPMD launch."""
    import ml_dtypes

    x = np.asarray(x, dtype=np.float32)
    g = np.asarray(g, dtype=np.float32).reshape(DIM)
    w_qkv = np.asarray(w_qkv, dtype=np.float32)
    w_out = np.asarray(w_out, dtype=np.float32)
    b_out = np.asarray(b_out, dtype=np.float32)

    in_maps = []
    for c in range(8):
        beta = c // 4
        h0 = 2 * (c % 4)
        h1 = h0 + 1
        x4 = (x[beta].reshape(DIM, N) / 4.0).astype(np.float32)
        # w_qkv rows: q block [0:512], k block [512:1024], v block [1024:1536]
        qr = np.r_[h0 * DH : (h0 + 1) * DH, h1 * DH : (h1 + 1) * DH]
        wq = w_qkv[qr]            # [128, DIM]
        wk = w_qkv[DIM + qr]      # [128, DIM]
        wvv = w_qkv[2 * DIM + qr]  # [128, DIM]
        gw = (g[None, :] * 4.0).astype(np.float32)
        # fold the attention 1/8 scale into wq so q2 and k2 share s_bc
        wqk = np.concatenate([wq * gw / 8.0, wk * gw], axis=0).T.copy()  # [DIM, 256]
        wvt = (wvv * gw).T.astype(ml_dtypes.bfloat16)  # [DIM, 128]
        wp = np.zeros((65, 2, DIM), dtype=np.float32)
        wp[0:64, 0, :] = w_out[:, h0 * DH : (h0 + 1) * DH].T
        wp[0:64, 1, :] = w_out[:, h1 * DH : (h1 + 1) * DH].T
        wp[64, :, :] = b_out[None, :] / 8.0
        in_maps.append(
            {
                "xbin": np.ascontiguousarray(x4.astype(ml_dtypes.bfloat16)),
                "wqk": np.ascontiguousarray(wqk.astype(ml_dtypes.bfloat16)),
                "wv": np.ascontiguousarray(wvt),
                "wp": wp.astype(ml_dtypes.float8_e4m3),
            }
        )
    return in_maps


def run_spmd(in_maps, trace=False):
    from concourse.bass_utils import run_bass_kernel_spmd

    nc = _get_program()
    return run_bass_kernel_spmd(nc, in_maps, list(range(8)), trace=trace)


def combine(results, x):
    x = np.asarray(x, dtype=np.float32)
    y = np.zeros((B, DIM, N), dtype=np.float32)
    for c in range(8):
        y[c // 4] += results[c]["y"]
    return y.reshape(B, DIM, HWS, HWS)


def kernel(x, g, w_qkv, w_out, b_out):
    in_maps = make_in_maps(x, g, w_qkv, w_out, b_out)
    res = run_spmd(in_maps)
    return combine(res.results, x)


# revision 32
# speedup vs baseline: 1.2636x; 1.1616x over previous
"""Trainium2 Bass kernel for nn_Attention_62861141344964.

Full-input contract: kernel(**inputs) takes the unsharded inputs and returns
the full-shape output. Internally shards across 8 NeuronCores as
(batch, head-pair): core c handles batch c//4 and heads {2*(c%4), 2*(c%4)+1}.

Per-core pipeline (ACT-exp is the bottleneck engine; everything else is
arranged around keeping its exp stream dense):
  - prologue per n-block: x DMA -> xsq(fp8, DVE) -> sumsq (fp8 DoubleRow
    matmul) -> DMA round-trip to transposed layout -> rsqrt via DVE bit-trick
    + 2 Newton steps (no ACT sqrt, so ACT runs exp only, one table load) ->
    broadcast (Pool) -> q/k projections (f32r) -> q2/k2 scaled (DVE).
  - attention: sim matmuls f32r -> exp on ACT with bias -2 emitting fp8e4
    directly -> AV as fp8 DoubleRow over j-tile pairs (0.5 cyc/row, two
    j-tiles per instruction) with plain-fp8 orphans; denominator rides row 0
    of vT/av (ones trick).
  - tail per i-block: reciprocal(den) -> Pool partition_broadcast -> DVE
    normalize into `on` (fp8) -> output projection as one fp8-DoubleRow
    matmul per 128-chunk (both heads contracted together) -> residual add on
    Pool -> DMA out.
The host folds g*sqrt(c) (and q's 1/8) into the weights and sums the 4
partial outputs per batch.
"""

import sys

sys.path.insert(0, "/opt/trn_rl_repo")

import numpy as np

HEADS = 8
DH = 64
DIM = 512
B = 2
HWS = 48
N = HWS * HWS  # 2304
KT = 4  # k-tiles of 128 over DIM
JT = 18  # j-tiles of 128 over N
NBLKS = [(0, 512), (512, 512), (1024, 512), (1536, 512), (2048, 256)]
MAGIC_SQRT = 0x1FBD1DF5
SQRT32 = 5.656854249492381

_CACHE = {}


def _build_program(debug=False):
    import concourse.bass as bass  # noqa: F401
    import concourse.mybir as mybir
    import concourse.tile as tile
    from concourse import bacc

    f32 = mybir.dt.float32
    f32r = mybir.dt.float32r
    bf16 = mybir.dt.bfloat16
    f8 = mybir.dt.float8e4
    i32 = mybir.dt.int32
    AF = mybir.ActivationFunctionType
    OP = mybir.AluOpType
    DRM = mybir.MatmulPerfMode.DoubleRow

    nc = bacc.Bacc("TRN2", target_bir_lowering=False, debug=False, num_devices=8)

    xb_d = nc.dram_tensor("xbin", [DIM, N], bf16, kind="ExternalInput").ap()
    wqk_d = nc.dram_tensor("wqk", [DIM, 256], bf16, kind="ExternalInput").ap()
    wv_d = nc.dram_tensor("wv", [DIM, 128], bf16, kind="ExternalInput").ap()
    wp_d = nc.dram_tensor("wp", [65, 2, DIM], f8, kind="ExternalInput").ap()
    y_d = nc.dram_tensor("y", [DIM, N], f32, kind="ExternalOutput").ap()
    dbg = {}
    if debug:
        for nm, shp, dt in [("q2", [128, N], f32), ("k2", [128, N], f32),
                            ("vT", [128, JT, 144], f32), ("on", [65, 2, N], f32),
                            ("s_bc", [128, N], f32), ("s_colT", [128, JT], f32)]:
            dbg[nm] = nc.dram_tensor("dbg_" + nm, shp, dt, kind="ExternalOutput").ap()

    with tile.TileContext(nc) as tc:
        big = tc.alloc_tile_pool(name="big", bufs=1)
        work = tc.alloc_tile_pool(name="work", bufs=2)
        rsq = tc.alloc_tile_pool(name="rsq", bufs=6)
        pg = tc.alloc_tile_pool(name="pg", bufs=1, space="PSUM")
        pav = tc.alloc_tile_pool(name="pav", bufs=4, space="PSUM")

        # ---------- persistent tiles ----------
        xb = big.tile([128, KT, N], bf16)
        q2 = big.tile([128, N], f32r)
        k2 = big.tile([128, N], f32r)
        s_bc = big.tile([128, N], f32)
        s_row = big.tile([1, N], f32)
        t_colT = big.tile([128, JT], f32)
        s_colT = big.tile([128, JT], f32)
        vT = big.tile([128, JT, 144], f8)
        on = big.tile([65, 2, N], f8)
        wqk_s = big.tile([128, KT, 256], bf16)
        wv_s = big.tile([128, KT, 128], bf16)
        wp_s = big.tile([65, 2, DIM], f8)
        ones8 = big.tile([128, 2, 16], f8)
        bias_m2 = big.tile([128, 1], f32)
        e64 = big.tile([128, 128], f32r)  # row 64 = ones: PE partition-bcast of row 64
        den_pad = [big.tile([128, 512], f32r, name="den_pad0"),
                   big.tile([128, 512], f32r, name="den_pad1")]


        nc.gpsimd.memset(ones8[:], 1.0)
        nc.vector.memset(bias_m2[:], -2.0)
        nc.gpsimd.memset(vT[:, :, 64:65], 1.0)
        nc.gpsimd.memset(vT[:, :, 136:137], 1.0)
        nc.vector.memset(e64[:].bitcast(f32), 0.0)
        nc.vector.memset(e64[64:65, :].bitcast(f32), 1.0)

        # x is shipped ONLY as bf16 (the cost model's DMA engines serialize
        # all transfers, so halving startup bytes halves the lead-in). wqk
        # leads so the block-0 projections can start immediately.
        xb_r = xb_d.rearrange("(a p) n -> p a n", p=128)
        nc.scalar.dma_start(wqk_s[:], wqk_d.rearrange("(a p) m -> p a m", p=128))
        o, w = NBLKS[0]
        nc.sync.dma_start(xb[:, :, o : o + w], xb_r[:, :, o : o + w])
        nc.scalar.dma_start(wv_s[:], wv_d.rearrange("(a p) m -> p a m", p=128))
        nc.scalar.dma_start(wp_s[:], wp_d)
        for (o, w) in NBLKS[1:]:
            nc.sync.dma_start(xb[:, :, o : o + w], xb_r[:, :, o : o + w])

        # ---------- prologue jobs (per n-block) ----------
        # The per-token norm scale s = sqrt(32/sumsq) is needed in TWO
        # layouts: as a row (-> partition_broadcast -> s_bc, scaling q2/k2
        # along the free axis) and transposed (s_colT, per-partition scalar
        # for the vT scaling). Computing sumsq in both orientations on the
        # PE (ones-vector matmuls) and running the rsqrt bit-trick + Newton
        # on each avoids any DMA transpose round-trip.
        def pro_sumsq(bi):
            o, w = NBLKS[bi]
            c0, cw = o // 128, w // 128

            def job():
                xsq = work.tile([128, KT, 512], f8, tag="xsq", name=f"xsq_{bi}")
                ps = pav.tile([1, 512], f32, tag="avy", name=f"ps_{bi}")
                for k in range(2):
                    nc.vector.tensor_tensor(
                        xsq[:, 2 * k : 2 * k + 2, :w],
                        xb[:, 2 * k : 2 * k + 2, o : o + w],
                        xb[:, 2 * k : 2 * k + 2, o : o + w], OP.mult,
                    )
                    nc.tensor.matmul(
                        ps[:, :w], ones8[:, :, 0:1], xsq[:, 2 * k : 2 * k + 2, :w],
                        start=(k == 0), stop=(k == 1), perf_mode=DRM,
                    )
                pt = pav.tile([128, 512], f32, tag="avy", name=f"pt_{bi}")
                for tt in range(cw):
                    for k in range(2):
                        nc.tensor.matmul(
                            pt[:, tt : tt + 1],
                            xsq[:, 2 * k : 2 * k + 2, tt * 128 : (tt + 1) * 128],
                            ones8[:, :, 0:1],
                            start=(tt == 0 and k == 0),
                            stop=(tt == cw - 1 and k == 1),
                            perf_mode=DRM,
                            skip_group_check=True,
                        )
                # row rsqrt: s_row = sqrt(32/t), seed from bits of 1/t,
                # one Newton step (0.2% worst case). t is copied to SBUF
                # first so the ps PSUM bank frees immediately.
                r0 = work.tile([1, 512], f32, tag="r0", name=f"r0_{bi}")
                r1 = work.tile([1, 512], f32, tag="r1", name=f"r1_{bi}")
                r2 = work.tile([1, 512], f32, tag="r2", name=f"r2_{bi}")
                t_sb = work.tile([1, 512], f32, tag="tsb", name=f"tsb_{bi}")
                nc.vector.tensor_copy(t_sb[:, :w], ps[:, :w])
                nc.vector.reciprocal(r0[:, :w], t_sb[:, :w])
                nc.vector.tensor_scalar(
                    r0[:, :w].bitcast(i32), r0[:, :w].bitcast(i32), 1, None,
                    OP.logical_shift_right,
                )
                nc.vector.tensor_scalar(
                    r0[:, :w].bitcast(i32), r0[:, :w].bitcast(i32), MAGIC_SQRT,
                    None, OP.add,
                )
                nc.vector.tensor_tensor(r1[:, :w], r0[:, :w], r0[:, :w], OP.mult)
                nc.vector.tensor_tensor(r2[:, :w], r1[:, :w], t_sb[:, :w], OP.mult)
                nc.vector.tensor_scalar(
                    r2[:, :w], r2[:, :w], -0.5 * SQRT32, 1.5 * SQRT32, OP.mult, OP.add
                )
                nc.vector.tensor_tensor(s_row[:, o : o + w], r2[:, :w], r0[:, :w], OP.mult)
                nc.gpsimd.partition_broadcast(s_bc[:, o : o + w], s_row[:, o : o + w])
                # transposed rsqrt for s_colT (tiny frees; two Newton steps)
                tc_ = rsq.tile([128, 4], f32, tag="rsq", name=f"tc_{bi}")
                nc.vector.tensor_copy(tc_[:, :cw], pt[:, :cw])
                y0 = rsq.tile([128, 4], f32, tag="rsq", name=f"y0_{bi}")
                aa = rsq.tile([128, 4], f32, tag="rsq", name=f"aa_{bi}")
                bb = rsq.tile([128, 4], f32, tag="rsq", name=f"bb_{bi}")
                nc.vector.reciprocal(y0[:, :cw], tc_[:, :cw])
                nc.vector.tensor_scalar(
                    y0[:, :cw].bitcast(i32), y0[:, :cw].bitcast(i32), 1, None,
                    OP.logical_shift_right,
                )
                nc.vector.tensor_scalar(
                    y0[:, :cw].bitcast(i32), y0[:, :cw].bitcast(i32), MAGIC_SQRT,
                    None, OP.add,
                )
                nc.vector.tensor_tensor(aa[:, :cw], y0[:, :cw], y0[:, :cw], OP.mult)
                nc.vector.tensor_tensor(bb[:, :cw], aa[:, :cw], tc_[:, :cw], OP.mult)
                nc.vector.tensor_scalar(
                    aa[:, :cw], bb[:, :cw], -0.5 * SQRT32, 1.5 * SQRT32, OP.mult, OP.add
                )
                nc.vector.tensor_tensor(
                    s_colT[:, c0 : c0 + cw], aa[:, :cw], y0[:, :cw], OP.mult
                )
            return job

        def pro_k(bi):
            o, w = NBLKS[bi]

            def job():
                pk = pav.tile([128, 512], f32, tag="avy", name=f"pk_{bi}")
                for kt in range(KT):
                    nc.tensor.matmul(
                        pk[:, :w], wqk_s[:, kt, 128:256], xb[:, kt, o : o + w],
                        start=(kt == 0), stop=(kt == KT - 1),
                    )
                nc.vector.tensor_tensor(
                    k2[:, o : o + w], pk[:, :w], s_bc[:, o : o + w], OP.mult
                )
            return job

        def pro_q(bi):
            o, w = NBLKS[bi]

            def job():
                pq = pav.tile([128, 512], f32, tag="avy", name=f"pq_{bi}")
                for kt in range(KT):
                    nc.tensor.matmul(
                        pq[:, :w], wqk_s[:, kt, 0:128], xb[:, kt, o : o + w],
                        start=(kt == 0), stop=(kt == KT - 1),
                    )
                nc.vector.tensor_tensor(
                    q2[:, o : o + w], pq[:, :w], s_bc[:, o : o + w], OP.mult
                )
            return job

        def vt_job(jt):
            def job():
                pv = pav.tile([128, 512], f32, tag="avy", name=f"pv_{jt}")
                for kt in range(KT):
                    nc.tensor.matmul(
                        pv[:, :128], xb[:, kt, jt * 128 : (jt + 1) * 128],
                        wv_s[:, kt, :], start=(kt == 0), stop=(kt == KT - 1),
                    )
                nc.vector.tensor_scalar_mul(
                    vT[:, jt, 0:64], pv[:, 0:64], s_colT[:, jt : jt + 1]
                )
                nc.vector.tensor_scalar_mul(
                    vT[:, jt, 72:136], pv[:, 64:128], s_colT[:, jt : jt + 1]
                )
            return job

        # job queue: prologue for blocks 1.. interleaved with vT jobs, popped
        # during the attention waves (block 0's prologue is emitted eagerly)
        jobs = []
        pro_done = [False] * len(NBLKS)
        vt_done = [False] * JT

        def mark_pro(bi):
            def f():
                pro_done[bi] = True
            return f

        def mark_vt(jt):
            def f():
                vt_done[jt] = True
            return f

        for fn in (pro_sumsq(0), pro_k(0), pro_q(0)):
            fn()
        pro_done[0] = True

        def late_memsets():
            nc.gpsimd.memset(on[64:65, :, :], 1.0)
            nc.gpsimd.memset(den_pad[0][:].bitcast(f32), 0.0)
            nc.gpsimd.memset(den_pad[1][:].bitcast(f32), 0.0)

        for bi in range(1, len(NBLKS)):
            jobs += [pro_sumsq(bi), pro_k(bi),
                     (pro_q(bi), mark_pro(bi))]
            lo = 4 * (bi - 1)
            jobs += [(vt_job(jt), mark_vt(jt)) for jt in range(lo, min(lo + 4, JT))]
        jobs.append(late_memsets)
        jobs += [(vt_job(jt), mark_vt(jt)) for jt in range(16, JT)]

        def pop_job():
            j = jobs.pop(0)
            if isinstance(j, tuple):
                j[0]()
                j[1]()
            else:
                j()

        def ensure_vt(jt):
            while not vt_done[jt]:
                pop_job()

        def ensure_block(bi):
            while not pro_done[bi]:
                pop_job()

        pwav = tc.alloc_tile_pool(name="pwav", bufs=10)
        ywork = tc.alloc_tile_pool(name="ywork", bufs=3)

        # ---------- wave plans ----------
        # full blocks (w=512): alternating G3/G2 tiles of (head, [jts]);
        # 3-waves carry a DR pair + an orphan, 2-waves a DR pair.
        # uniform waves of DR pairs: two psum groups double-buffer (4 banks)
        # leaving FOUR avy banks, so two prologue chains stay in flight and
        # every AV instruction is a DoubleRow pair (no plain-fp8 orphans)
        def wave_plan_512():
            plan = []  # (gtag, size, groups=[(head, jts)])
            a = list(range(JT))
            b = list(range(JT))
            for i in range(18):
                head, srcq = (0, a) if i % 2 == 0 else (1, b)
                jts = [srcq.pop(0), srcq.pop(0)]
                plan.append(("G2A" if i % 2 == 0 else "G2B", 2, [(head, jts)]))
            assert not a and not b, (a, b)
            return plan

        def wave_plan_256():
            plan = []
            a = list(range(JT))
            b = list(range(JT))
            for i in range(9):
                if i < 8:
                    head, srcq = (0, a) if i % 2 == 0 else (1, b)
                    groups = [(head, [srcq.pop(0) for _ in range(4)])]
                else:
                    groups = [(0, [a.pop(0), a.pop(0)]), (1, [b.pop(0), b.pop(0)])]
                plan.append(("G2A" if i % 2 == 0 else "G2B", 4, groups))
            assert not a and not b, (a, b)
            return plan

        def emit_sims(g, o, w, groups):
            slot = 0
            for head, jts in groups:
                for jt in jts:
                    nc.tensor.matmul(
                        g[:, slot, :],
                        k2[64 * head : 64 * (head + 1), jt * 128 : (jt + 1) * 128],
                        q2[64 * head : 64 * (head + 1), o : o + w],
                        start=True, stop=True,
                    )
                    slot += 1

        def emit_avs(psb, w, groups, av, avflags):
            slot = 0
            for head, jts in groups:
                vbase = 72 * head
                flags = avflags[head]
                for i in range(0, len(jts), 2):
                    assert jts[i + 1] == jts[i] + 1
                    nc.tensor.matmul(
                        av[head][:, :w],
                        vT[:, jts[i] : jts[i] + 2, vbase : vbase + 65],
                        psb[:, slot : slot + 2, :],
                        start=flags["first"], stop=flags["remaining"] == 2,
                        perf_mode=DRM, skip_group_check=True,
                    )
                    slot += 2
                    flags["remaining"] -= 2
                    flags["first"] = False

        def make_tail_norm(ib, o, w, av, h):
            def tail():
                with nc.allow_low_precision(reason="1/den broadcast via f32r matmul"):
                    nc.vector.reciprocal(den_pad[h][64:65, :w], av[h][64:65, :w])
                dbc = pav.tile([128, 512], f32, tag="avy", name=f"dbc_{ib}_{h}")
                nc.tensor.matmul(
                    dbc[:, :w], e64[:], den_pad[h][:, :w], start=True, stop=True
                )
                rb = work.tile([128, 512], f32, tag="rb", name=f"rb_{ib}_{h}")
                nc.vector.tensor_copy(rb[:, :w], dbc[:, :w])
                nc.vector.tensor_tensor(
                    on[0:64, h, o : o + w], av[h][0:64, :w], rb[0:64, :w], OP.mult
                )
            return tail

        def make_tail_proj(ib, o, w):
            def tail():
                ysb = ywork.tile([128, KT, 512], f32, tag="y", name=f"ysb_{ib}")
                for ot in range(KT):
                    py = pav.tile([128, 512], f32, tag="avy", name=f"py_{ib}_{ot}")
                    nc.tensor.matmul(
                        py[:, :w], wp_s[:, :, ot * 128 : (ot + 1) * 128],
                        on[:, :, o : o + w],
                        start=True, stop=True, perf_mode=DRM,
                    )
                    nc.vector.tensor_tensor(
                        ysb[:, ot, :w], py[:, :w], xb[:, ot, o : o + w], OP.add
                    )
                nc.sync.dma_start(
                    y_d.rearrange("(a p) n -> p a n", p=128)[:, :, o : o + w],
                    ysb[:, :, :w],
                )
            return tail

        # ---------- attention ----------
        deferred = []
        for ib, (o, w) in enumerate(NBLKS):
            ensure_block(ib)
            plan = wave_plan_512() if w == 512 else wave_plan_256()
            av = [
                pav.tile([65, 512], f32, tag="avy", name=f"av0_{ib}"),
                pav.tile([65, 512], f32, tag="avy", name=f"av1_{ib}"),
            ]
            avflags = [{"first": True, "remaining": JT}, {"first": True, "remaining": JT}]
            pending = None
            for wv_i, (gtag, size, groups) in enumerate(plan):
                g = pg.tile([128, size, w], f32, tag=gtag, name=f"g_{ib}_{wv_i}")
                emit_sims(g, o, w, groups)
                p_sb = pwav.tile([128, size, w], f8, tag="P", name=f"p_{ib}_{wv_i}")
                nc.scalar.activation(p_sb[:], g[:], AF.Exp, bias=bias_m2[:])
                if deferred and wv_i == 0:
                    deferred.pop(0)()
                waves = [pending, (p_sb, groups)] if pending else [(p_sb, groups)]
                if wv_i < len(plan) - 1:
                    pending = waves.pop()
                for psb_j, groups_j in waves:
                    for _, jts_j in groups_j:
                        for jt in jts_j:
                            ensure_vt(jt)
                    for _ in range(2):
                        if jobs:
                            pop_job()
                    emit_avs(psb_j, w, groups_j, av, avflags)
            make_tail_norm(ib, o, w, av, 0)()
            make_tail_norm(ib, o, w, av, 1)()
            deferred = [make_tail_proj(ib, o, w)]
        while jobs:
            pop_job()
        for fn in deferred:
            fn()

        if debug:
            nc.sync.dma_start(dbg["q2"], q2[:].bitcast(f32))
            nc.sync.dma_start(dbg["k2"], k2[:].bitcast(f32))
            vtf = big.tile([128, JT, 144], f32)
            nc.vector.tensor_copy(vtf[:], vT[:])
            nc.sync.dma_start(dbg["vT"], vtf[:])
            onf = big.tile([65, 2, N], f32)
            nc.vector.tensor_copy(onf[:], on[:])
            nc.sync.dma_start(dbg["on"], onf[:])
            nc.sync.dma_start(dbg["s_bc"], s_bc[:])
            nc.sync.dma_start(dbg["s_colT"], s_colT[:])
        for pool in (ywork, pwav, pav, pg, rsq, work, big):
            pool.release()

    nc.compile()
    return nc


def _get_program():
    if "nc" not in _CACHE:
        _CACHE["nc"] = _build_program()
    return _CACHE["nc"]


def make_in_maps(x, g, w_qkv, w_out, b_out):
    """Build the per-core input dicts for the SPMD launch."""
    import ml_dtypes

    x = np.asarray(x, dtype=np.float32)
    g = np.asarray(g, dtype=np.float32).reshape(DIM)
    w_qkv = np.asarray(w_qkv, dtype=np.float32)
    w_out = np.asarray(w_out, dtype=np.float32)
    b_out = np.asarray(b_out, dtype=np.float32)

    in_maps = []
    for c in range(8):
        beta = c // 4
        h0 = 2 * (c % 4)
        h1 = h0 + 1
        x4 = (x[beta].reshape(DIM, N) / 4.0).astype(np.float32)
        # w_qkv rows: q block [0:512], k block [512:1024], v block [1024:1536]
        qr = np.r_[h0 * DH : (h0 + 1) * DH, h1 * DH : (h1 + 1) * DH]
        wq = w_qkv[qr]            # [128, DIM]
        wk = w_qkv[DIM + qr]      # [128, DIM]
        wvv = w_qkv[2 * DIM + qr]  # [128, DIM]
        gw = (g[None, :] * 4.0).astype(np.float32)
        # fold the attention 1/8 scale into wq so q2 and k2 share s_bc
        wqk = np.concatenate([wq * gw / 8.0, wk * gw], axis=0).T.copy()  # [DIM, 256]
        wvt = (wvv * gw).T.astype(ml_dtypes.bfloat16)  # [DIM, 128]
        wp = np.zeros((65, 2, DIM), dtype=np.float32)
        wp[0:64, 0, :] = w_out[:, h0 * DH : (h0 + 1) * DH].T
        wp[0:64, 1, :] = w_out[:, h1 * DH : (h1 + 1) * DH].T
        wp[64, :, :] = b_out[None, :] / 8.0
        in_maps.append(
            {
                "xbin": np.ascontiguousarray(x4.astype(ml_dtypes.bfloat16)),
                "wqk": np.ascontiguousarray(wqk.astype(ml_dtypes.bfloat16)),
                "wv": np.ascontiguousarray(wvt),
                "wp": wp.astype(ml_dtypes.float8_e4m3),
            }
        )
    return in_maps


def run_spmd(in_maps, trace=False):
    from concourse.bass_utils import run_bass_kernel_spmd

    nc = _get_program()
    return run_bass_kernel_spmd(nc, in_maps, list(range(8)), trace=trace)


def combine(results, x):
    x = np.asarray(x, dtype=np.float32)
    y = np.zeros((B, DIM, N), dtype=np.float32)
    for c in range(8):
        y[c // 4] += results[c]["y"]
    return y.reshape(B, DIM, HWS, HWS)


def kernel(x, g, w_qkv, w_out, b_out):
    in_maps = make_in_maps(x, g, w_qkv, w_out, b_out)
    res = run_spmd(in_maps)
    return combine(res.results, x)
